# revision 3
# baseline (speedup 1.0000x reference)
"""Trainium2 Bass kernel for MiLoLinear: out = x @ (dequant4(W_q) + U@V).T + bias.

Sharding: column-parallel over the 172 dequant groups (gq). Cores 0-3 take 22
groups, cores 4-7 take 21 (+1 zero pad) -> every core computes 1408 output
columns (64 r x 22 gq) of the [512, 11008] output; the host gathers/reorders.

Math per core (all exact rewrites of the reference):
  o = r*172 + gq  (r in [0,64)),  local col j = r*22 + g
  out[s,o] = sum_c x[s,c]*Q[o,c]*scale[gq,c]            (PE, bf16)
           - sum_c x[s,c]*(scale*zero)[gq,c]            (folded: T-rows correction)
           + (x @ V.T) @ U.T + bias                      (folded: y-rows + ones row)
The corrections ride the PE as 55 extra contraction rows computed by a
phase-A matmul stream (zv stationary) interleaved into sweep 1.

Device pipeline (per core):
  - wq arrives host-expanded to u8 (one byte per 4-bit weight); dequant is a
    single DVE tensor_tensor per k-tile: wbf[c, r*22+g] = u8 * scale[c, g]
    with the scale read through a stride-0 broadcast AP from a compact
    [128, 22] tile (no 32x-replicated scale traffic).
  - Output computed in two column sweeps so PSUM (8 banks) suffices:
      sweep1: st widths (1024,1024,1024,512), + phase-A MM per t
      sweep2: st widths (384,384,384,896)
    drains (psum -> sbuf bf16 -> DMA) overlap the next sweep's compute.
"""

import sys

for _p in ("/opt/trn_rl_repo", "/root/.axon_site/_ro/trn_rl_repo"):
    if _p not in sys.path:
        sys.path.append(_p)

import numpy as np
import ml_dtypes

import concourse.bass as bass
import concourse.tile as tile
from concourse import bacc, mybir
from concourse.bass_utils import run_bass_kernel_spmd

OUT_F, IN_F, GROUP = 11008, 4096, 64
G = OUT_F * IN_F // GROUP            # 704512
GQ = G // IN_F                       # 172 groups along out axis
S = 512                              # rows of x
NCORES = 8
GQL = 22                             # padded gq per core
NKT = IN_F // 128                    # 32 contraction tiles
OL = 64 * GQL                        # 1408 local output columns
NCORR = 55                           # 22 T-rows + 32 y-rows + 1 ones-row

W1 = (1024, 1024, 1024, 512)         # sweep-1 column width per s-tile
W2 = tuple(OL - w for w in W1)       # sweep-2 widths (384,384,384,896)

BF16 = ml_dtypes.bfloat16

# gq ownership: cores 0-3 -> 22 groups, cores 4-7 -> 21 (+ pad)
_SIZES = [22, 22, 22, 22, 21, 21, 21, 21]
_STARTS = np.cumsum([0] + _SIZES[:-1]).tolist()


def _core_gqs(k):
    gqs = list(range(_STARTS[k], _STARTS[k] + _SIZES[k]))
    return gqs + [-1] * (GQL - len(gqs))


def _halves(w):
    return [(a, min(a + 512, w)) for a in range(0, w, 512)]


def _build_program():
    nc = bacc.Bacc("TRN2", target_bir_lowering=False, debug=False)
    dt = mybir.dt

    NWCH = 16                        # wq DMA chunks (2 k-tiles each)
    TPW = NKT // NWCH                # k-tiles per wq chunk = 2
    wq_in = nc.declare_dram_parameter("wq", [NWCH, 128, TPW * OL], dt.uint8,
                                      isOutput=False)
    sc_in = nc.declare_dram_parameter("sc", [128, NKT * GQL], dt.bfloat16,
                                      isOutput=False)
    xt_in = nc.declare_dram_parameter("xt", [128, NKT * S], dt.bfloat16,
                                      isOutput=False)
    zv_in = nc.declare_dram_parameter("zv", [128, NKT * (NCORR - 1)],
                                      dt.bfloat16, isOutput=False)
    cr_in = nc.declare_dram_parameter("cr", [NCORR, OL], dt.bfloat16,
                                      isOutput=False)
    out_d = nc.declare_dram_parameter("out", [4, 128, OL], dt.bfloat16,
                                      isOutput=True)

    with tile.TileContext(nc) as tc:
        with (
            tc.tile_pool(name="const", bufs=1) as cpool,
            tc.tile_pool(name="wq", bufs=3) as wqp,
            tc.tile_pool(name="out", bufs=1) as outp,
            tc.tile_pool(name="ps", bufs=1, space="PSUM") as psp,
        ):
            xt = cpool.tile([128, NKT * S], dt.bfloat16)
            zv = cpool.tile([128, NKT * (NCORR - 1)], dt.bfloat16)
            sc = cpool.tile([128, NKT * GQL], dt.bfloat16)
            cr = cpool.tile([NCORR, OL], dt.bfloat16)
            corr = cpool.tile([NCORR, S], dt.bfloat16)
            wbf = cpool.tile([128, NKT * OL], dt.bfloat16)

            # ---- DMAs: sync carries sc/zv/xt/cr; scalar streams wq ----
            nc.sync.dma_start(sc[:], sc_in[:])
            nc.sync.dma_start(zv[:], zv_in[:])
            for i in range(8):
                t = i * 4
                nc.sync.dma_start(xt[:, t * S:(t + 4) * S],
                                  xt_in[:, t * S:(t + 4) * S])
            nc.sync.dma_start(cr[:], cr_in[:])
            wq_t = []
            for ch in range(NWCH):
                wqc = wqp.tile([128, TPW * OL], dt.uint8, tag="wq",
                               name=f"wq{ch}")
                nc.scalar.dma_start(wqc[:], wq_in[ch])
                for q in range(TPW):
                    wq_t.append(wqc[:, q * OL:(q + 1) * OL])

            # ---- psum tiles (8 banks total: big 3x2 + sm 2x1) ----
            pa = psp.tile([NCORR - 1, S], dt.float32, tag="sm", bufs=2,
                          padded_shape=[128, S], name="pa")
            ps1 = []
            for st in range(3):
                ps1.append(psp.tile([128, W1[st]], dt.float32, tag="big",
                                    bufs=3, name=f"ps1_{st}"))
            ps1.append(psp.tile([128, W1[3]], dt.float32, tag="sm", bufs=2,
                                name="ps1_3"))

            # ---- init + PE warmup (HAM clock) while input DMAs stream ----
            nc.vector.memset(corr[:], 1.0)       # row 54 stays the ones-row
            for _ in range(6):
                nc.tensor.matmul(pa[:], corr[:, 0:NCORR - 1], corr[:],
                                 start=True, stop=True)

            # ---- DVE dequant stream: one broadcast mult per k-tile ----
            # wbf[c, j] = u8(Q) * scale[c, g],  j = r*22+g, scale stride-0
            # over r via a [0,64] AP dim.
            for t in range(NKT):
                wap = wbf[:, t * OL:(t + 1) * OL]
                qap = wq_t[t]
                sap = sc[:, t * GQL:(t + 1) * GQL]
                dst = bass.AP(wap.tensor, wap.offset,
                              [wap.ap[0], [GQL, 64], [1, GQL]])
                src0 = bass.AP(qap.tensor, qap.offset,
                               [qap.ap[0], [GQL, 64], [1, GQL]])
                src1 = bass.AP(sap.tensor, sap.offset,
                               [sap.ap[0], [0, 64], [1, GQL]])
                nc.vector.tensor_tensor(dst, src0, src1,
                                        op=mybir.AluOpType.mult)

            # ---- sweep 1: phase-A + st columns [0, W1[st]) ----
            for t in range(NKT):
                nc.tensor.matmul(
                    pa[:], zv[:, t * (NCORR - 1):(t + 1) * (NCORR - 1)],
                    xt[:, t * S:(t + 1) * S],
                    start=(t == 0), stop=(t == NKT - 1),
                )
                for st in range(4):
                    lhs = xt[:, t * S + st * 128: t * S + (st + 1) * 128]
                    for a, b in _halves(W1[st]):
                        nc.tensor.matmul(
                            ps1[st][:, a:b], lhs,
                            wbf[:, t * OL + a:t * OL + b],
                            start=(t == 0), stop=False)

            # ---- epilogue 1: corr rows, then drain sweep-1 psum ----
            nc.vector.tensor_copy(corr[0:NCORR - 1, :], pa[:])
            for st in range(4):
                clhs = corr[:, st * 128:(st + 1) * 128]
                for a, b in _halves(W1[st]):
                    nc.tensor.matmul(ps1[st][:, a:b], clhs, cr[:, a:b],
                                     start=False, stop=True)
                ot = outp.tile([128, W1[st]], dt.bfloat16, tag=f"o1_{st}",
                               name=f"ot1_{st}")
                nc.scalar.copy(ot[:], ps1[st][:])
                nc.sync.dma_start(out_d[st][:, 0:W1[st]], ot[:])

            # ---- sweep 2: st columns [W1[st], 1408) ----
            ps2 = []
            for st in (0, 1):
                ps2.append(psp.tile([128, W2[st]], dt.float32, tag="sm",
                                    bufs=2, name=f"ps2_{st}"))
            ps2.append(psp.tile([128, W2[2]], dt.float32, tag="big", bufs=3,
                                name="ps2_2"))
            ps2.append(psp.tile([128, W2[3]], dt.float32, tag="big", bufs=3,
                                name="ps2_3"))
            for t in range(NKT):
                for st in range(4):
                    lhs = xt[:, t * S + st * 128: t * S + (st + 1) * 128]
                    for a, b in _halves(W2[st]):
                        nc.tensor.matmul(
                            ps2[st][:, a:b], lhs,
                            wbf[:, t * OL + W1[st] + a:t * OL + W1[st] + b],
                            start=(t == 0), stop=False)

            # ---- epilogue 2: corr rows, drain (big st3 first) ----
            for st in (3, 2, 0, 1):
                clhs = corr[:, st * 128:(st + 1) * 128]
                for a, b in _halves(W2[st]):
                    nc.tensor.matmul(ps2[st][:, a:b], clhs,
                                     cr[:, W1[st] + a:W1[st] + b],
                                     start=False, stop=True)
                ot = outp.tile([128, W2[st]], dt.bfloat16, tag=f"o2_{st}",
                               name=f"ot2_{st}")
                nc.scalar.copy(ot[:], ps2[st][:])
                nc.sync.dma_start(out_d[st][:, W1[st]:OL], ot[:])

    nc.compile()
    return nc


def _prep_inputs(x, W_q, scale, zero, U, V, bias):
    """Build the 8 per-core input maps (all host-side numpy)."""
    Wq_u8 = W_q.astype(np.uint8).reshape(32, GQ, IN_F)
    scale_g = scale.reshape(GQ, IN_F).astype(np.float32)
    zero_g = zero.reshape(GQ, IN_F).astype(np.float32)
    sz_g = scale_g * zero_g

    # xt[p, t*S+s] = x[s, t*128+p]  (contiguous per-partition DMA layout)
    xt = np.ascontiguousarray(
        x.T.reshape(NKT, 128, S).transpose(1, 0, 2).reshape(128, NKT * S)
    ).astype(BF16)

    in_maps = []
    o_maps = []
    for k in range(NCORES):
        gqs = _core_gqs(k)
        valid = np.array([g >= 0 for g in gqs])
        gq_idx = np.array([g if g >= 0 else 0 for g in gqs])

        # expanded u8 weights: Q[r64, g, c]; hi nibble -> rows 0..31
        A = Wq_u8[:, gq_idx, :].copy()
        A[:, ~valid, :] = 0
        Q = np.concatenate([(A >> 4) & 0xF, A & 0xF], axis=0)  # [64, 22, c]
        # wq_dev[ch, p, q*OL + r64*22 + g] with t = ch*2 + q
        wq_dev = np.ascontiguousarray(
            Q.transpose(2, 0, 1).reshape(NKT, 128, OL)
            .reshape(16, 2, 128, OL).transpose(0, 2, 1, 3)
        ).reshape(16, 128, 2 * OL)

        # compact scale: sc_dev[p, t*22+g] = scale[gq(g), t*128+p]
        Sg = scale_g[gq_idx].copy()
        Sg[~valid] = 0.0
        sc_dev = np.ascontiguousarray(
            Sg.T.reshape(NKT, 128, GQL).transpose(1, 0, 2)
        ).astype(BF16).reshape(128, NKT * GQL)

        # zv: [c, 22 sz-rows + 32 V-rows] per k-tile
        Zg = sz_g[gq_idx].copy()
        Zg[~valid] = 0.0
        zv_dev = np.ascontiguousarray(
            np.concatenate([Zg.T, V.T.astype(np.float32)], axis=1)
            .reshape(NKT, 128, NCORR - 1).transpose(1, 0, 2)
            .reshape(128, NKT * (NCORR - 1))
        ).astype(BF16)

        # local output column map: j = r64*22 + g -> global o
        r64 = np.arange(OL) // GQL
        gql = np.arange(OL) % GQL
        gq_glob = np.array(gqs)[gql]
        o_map = np.where(gq_glob >= 0, r64 * GQ + gq_glob, -1)
        o_maps.append(o_map)

        # correction moving rows: [-indicator(22); U_T(32); bias(1)]
        cr_dev = np.zeros((NCORR, OL), dtype=np.float32)
        ind = gql[None, :] == np.arange(GQL)[:, None]      # [22, 1408]
        cr_dev[:GQL] = np.where(ind, -1.0, 0.0)
        ok = o_map >= 0
        cr_dev[:GQL, ~ok] = 0.0
        cr_dev[GQL:GQL + 32, ok] = U[o_map[ok]].astype(np.float32).T
        cr_dev[NCORR - 1, ok] = bias[o_map[ok]].astype(np.float32)
        cr_dev = cr_dev.astype(BF16)

        in_maps.append({
            "wq": wq_dev, "sc": sc_dev, "xt": xt, "zv": zv_dev, "cr": cr_dev,
        })
    return in_maps, o_maps


_CACHE = {}


def kernel(x, W_q, scale, zero, U, V, bias):
    x = np.asarray(x)
    W_q = np.asarray(W_q)
    scale = np.asarray(scale)
    zero = np.asarray(zero)
    U = np.asarray(U)
    V = np.asarray(V)
    bias = np.asarray(bias)

    if "nc" not in _CACHE:
        _CACHE["nc"] = _build_program()
    nc = _CACHE["nc"]

    in_maps, o_maps = _prep_inputs(x, W_q, scale, zero, U, V, bias)
    res = run_bass_kernel_spmd(nc, in_maps, list(range(NCORES)))

    out = np.zeros((S, OUT_F), dtype=np.float32)
    for k in range(NCORES):
        oc = res.results[k]["out"].reshape(S, OL).astype(np.float32)
        ok = o_maps[k] >= 0
        out[:, o_maps[k][ok]] = oc[:, ok]
    return out
